# revision 12
# baseline (speedup 1.0000x reference)
"""Trainium2 Bass kernel for nn_Network_27994596835705 (GNN message passing).

Strategy (8 NeuronCores, SPMD):
  - Shard t-nodes (attention destinations) by batch: core c owns t-dst
    rows [500c, 500c+500) (= graph instances 2c, 2c+1) and x-rows
    [2000c, 2000c+2000).  Edges are routed by destination on the host
    (the random test edges are NOT block-diagonal), sorted into 16
    dst-chunks of 32 dst x 9 edge-tiles of 128 edges.
  - Per conv, per core: two SWDGE dma_gathers fetch bf16 source features
    for every edge in both layouts ([e,c] and [c,e]).  Attention logits
    are computed as  logits[e,(d,h)] = xs[e]·U[d,h]  with the Wk
    contraction pre-folded into the dst side (U = Wk_h^T q / sqrt(C)),
    so all per-edge tensors stay 128-dim.  A host-built fp8 negmask
    (-192 at non-matching (dst,h) columns) is pre-accumulated into PSUM
    via an identity matmul, exp() maps masked entries to exactly 0, and
    message aggregation + softmax denominators are plain PE matmuls.
    Segment-max subtraction is skipped (logits are bounded, softmax is
    shift-invariant; verified |logits| < 30 on the reference data).
  - Per layer: one bf16 AllGather each for the updated t-node and x-node
    feature tables (gather sources live in DRAM).
  - Dense t-side math runs in f32/f32r; the x-side MLP and all edge math
    run in bf16 with f32 accumulation.
"""
import os
import numpy as np

import concourse.bass as bass
import concourse.bacc as bacc
import concourse.mybir as mybir
import concourse.tile as tile
from concourse import bass_utils

import ml_dtypes
BF16 = ml_dtypes.bfloat16
FP8 = ml_dtypes.float8_e4m3

F32 = mybir.dt.float32
F32R = mybir.dt.float32r
BF = mybir.dt.bfloat16
F8 = mybir.dt.float8e4
I16 = mybir.dt.int16

P = 128
NCORES = 8
BATCH, NN, DN = 16, 1000, 4
RR, DR = 250, 8
D, H, C, L = 128, 4, 128, 3
NEG = -192.0

NDST = 500          # real t-dst per core
DSLOT = 512         # padded (4 n-tiles)
NXOWN = 2000        # real x rows per core
XSLOT = 2048        # padded (16 n-tiles)
NCHUNK = 16         # dst chunks per conv
CHD = 32            # dst per chunk
TPC = 9             # edge tiles per chunk (1152 slots; data max is ~1116)
NT = NCHUNK * TPC   # 144 edge tiles per conv
EP = NT * P         # 18432 edge slots
TTAB = NCORES * DSLOT
XTAB = NCORES * XSLOT


# ---------------------------------------------------------------------------
# host-side preprocessing
# ---------------------------------------------------------------------------

def _prep_graph(src, dst, core, src_reindex):
    """Route one edge list to one core's chunk/tile layout.

    Returns (idx int16 [128, EP//16], negmask fp8 [128, NT*128]).
    """
    sel = (dst >= NDST * core) & (dst < NDST * (core + 1))
    esrc = src[sel]
    edst = dst[sel] - NDST * core
    # dummy dst slots 500..511: one unmasked edge each so den > 0
    esrc = np.concatenate([esrc, np.zeros(DSLOT - NDST, esrc.dtype)])
    edst = np.concatenate([edst, np.arange(NDST, DSLOT, dtype=edst.dtype)])

    srcslot = np.zeros(EP, np.int64)
    nm = np.full((EP, P), NEG, np.float32)
    chunk = edst // CHD
    for k in range(NCHUNK):
        m = chunk == k
        n = int(m.sum())
        assert n <= TPC * P, f"chunk {k} overflow: {n}"
        base = k * TPC * P
        srcslot[base:base + n] = src_reindex(esrc[m])
        dloc = (edst[m] % CHD).astype(np.int64)
        cols = dloc[:, None] * H + np.arange(H)[None, :]
        nm[(base + np.arange(n))[:, None], cols] = 0.0

    assert srcslot.max() < 32768
    idx = srcslot.astype(np.int16)
    idx16 = np.tile(idx.reshape(EP // 16, 16).T, (8, 1)).copy()
    nm_dev = nm.reshape(NT, P, P).transpose(1, 0, 2).reshape(P, NT * P)
    return idx16, np.ascontiguousarray(nm_dev).astype(FP8)


def _prep_inputs(x, t, e_t_src, e_t_dst, e_x_src, e_x_dst, params):
    x = np.asarray(x, np.float32).reshape(BATCH * NN, DN)
    t = np.asarray(t, np.float32).reshape(BATCH * RR, DR)
    e_t_src = np.asarray(e_t_src, np.int64)
    e_t_dst = np.asarray(e_t_dst, np.int64)
    e_x_src = np.asarray(e_x_src, np.int64)
    e_x_dst = np.asarray(e_x_dst, np.int64)
    pf = lambda a: np.asarray(a, np.float32)

    t_reindex = lambda n: DSLOT * (n // NDST) + n % NDST
    x_reindex = lambda n: XSLOT * (n // NN // 2) + 1024 * ((n // NN) % 2) + n % NN

    sc = np.float32(1.0 / np.sqrt(C))

    # ---- static f32 blob
    ws_parts, ws_off = [], {}

    def put(name, arr):
        ws_off[name] = sum(a.shape[1] for a in ws_parts)
        ws_parts.append(np.ascontiguousarray(arr, np.float32))

    def pad128(a):
        out = np.zeros((P, a.shape[1]), np.float32)
        out[: a.shape[0]] = a
        return out

    wr_parts, wr_off = [], {}

    def putr(name, arr):
        wr_off[name] = sum(a.shape[1] for a in wr_parts)
        wr_parts.append(np.ascontiguousarray(arr, np.float32))

    putr("fct_w", pad128(pf(params["fc_t"]["W"])))
    putr("fcx_w", pad128(pf(params["fc_x"]["W"])))
    putr("fcout_w", pf(params["fc_out"]["W"]))
    for l, bp in enumerate(params["blocks"]):
        putr(f"wout{l}", pf(bp["out"]["W"]))
        put(f"gb{l}", np.tile(pf(bp["out"]["g"])[None, :], (P, 1)))
        put(f"bb{l}", np.tile(pf(bp["out"]["beta"])[None, :], (P, 1)))
        put(f"b1_{l}", pf(bp["fcx"]["b1"])[:, None])
    wstat = np.concatenate(ws_parts, axis=1)
    wstatr = np.concatenate(wr_parts, axis=1)

    # ---- per-layer f32 blob: Wq, WkT/sqrtC, Ws per conv
    wlay, wl_off = [], {}
    for l, bp in enumerate(params["blocks"]):
        parts, off = [], {}
        for v in ("tt", "xct"):
            cp = {k: pf(a) for k, a in bp[v].items()}
            off[f"wq_{v}"] = sum(a.shape[1] for a in parts)
            parts.append(cp["Wq"])
            wkT = np.concatenate(
                [cp["Wk"][:, h * C:(h + 1) * C].T * sc for h in range(H)], axis=1)
            off[f"wkT_{v}"] = sum(a.shape[1] for a in parts)
            parts.append(wkT)
            off[f"ws_{v}"] = sum(a.shape[1] for a in parts)
            parts.append(cp["Ws"])
        wl_off[l] = off
        wlay.append(np.concatenate(parts, axis=1).astype(np.float32))
    WLCOLS = wlay[0].shape[1]

    # ---- bf16 blob
    bf_parts, bf_off = [], {}

    def putbf(name, arr):
        bf_off[name] = sum(a.shape[1] for a in bf_parts)
        bf_parts.append(np.ascontiguousarray(arr, np.float32))

    for l, bp in enumerate(params["blocks"]):
        putbf(f"w1_{l}", pf(bp["fcx"]["W1"]))
        putbf(f"w2_{l}", pf(bp["fcx"]["W2"]))
        putbf(f"b2r_{l}", pad128(pf(bp["fcx"]["b2"])[None, :]))
        for v in ("tt", "xct"):
            putbf(f"wv4_{l}{v}", pf(bp[v]["Wv"]) / 4.0)
    wbf = np.concatenate(bf_parts, axis=1)

    # ---- bias rows f32 [1, ...]
    br_parts, br_off = [], {}

    def putbr(name, row):
        br_off[name] = sum(a.shape[1] for a in br_parts)
        br_parts.append(np.asarray(row, np.float32).reshape(1, -1))

    putbr("ones128", np.ones(128, np.float32))
    putbr("ones512", np.ones(512, np.float32))
    putbr("fct_b", pf(params["fc_t"]["b"]))
    putbr("fcx_b", pf(params["fc_x"]["b"]))
    putbr("fcout_b", pf(params["fc_out"]["b"]))
    for l, bp in enumerate(params["blocks"]):
        for v in ("tt", "xct"):
            bconv = pf(bp[v]["bs"]) + pf(bp[v]["bv"]).reshape(H, C).mean(0)
            putbr(f"bconv_{l}{v}", bconv)
            putbr(f"bq_{l}{v}", pf(bp[v]["bq"]))      # [1, 512]
        putbr(f"bout_{l}", pf(bp["out"]["b"]))
    brow = np.concatenate(br_parts, axis=1)

    offsets = dict(ws=ws_off, wr=wr_off, wl=wl_off, bf=bf_off, br=br_off,
                   WLCOLS=WLCOLS, WSCOLS=wstat.shape[1], WRCOLS=wstatr.shape[1],
                   WBCOLS=wbf.shape[1], BRCOLS=brow.shape[1])

    in_maps = []
    for c in range(NCORES):
        idx_t, nm_t = _prep_graph(e_t_src, e_t_dst, c, t_reindex)
        idx_x, nm_x = _prep_graph(e_x_src, e_x_dst, c, x_reindex)
        tT = np.zeros((8, DSLOT), np.float32)
        tT[:, :NDST] = t[NDST * c:NDST * (c + 1)].T
        xT = np.zeros((4, XSLOT), np.float32)
        xo = x[NXOWN * c:NXOWN * (c + 1)].reshape(2, NN, DN)
        xT[:, 0:NN] = xo[0].T
        xT[:, 1024:1024 + NN] = xo[1].T
        m = {
            "idx_t": idx_t, "idx_x": idx_x,
            "nm_t": nm_t, "nm_x": nm_x,
            "tT": tT, "xT": xT,
            "wstat": wstat, "wstatr": wstatr, "wbf": wbf.astype(BF16),
            "brow": brow,
        }
        for l in range(L):
            m[f"wlay{l}"] = wlay[l]
        in_maps.append(m)
    return in_maps, offsets


# ---------------------------------------------------------------------------
# device program
# ---------------------------------------------------------------------------

def build_program(offs):
    nc = bacc.Bacc("TRN2", target_bir_lowering=False, debug=False,
                   num_devices=NCORES)
    d_idx_t = nc.dram_tensor("idx_t", [P, EP // 16], I16, kind="ExternalInput")
    d_idx_x = nc.dram_tensor("idx_x", [P, EP // 16], I16, kind="ExternalInput")
    d_nm_t = nc.dram_tensor("nm_t", [P, NT * P], F8, kind="ExternalInput")
    d_nm_x = nc.dram_tensor("nm_x", [P, NT * P], F8, kind="ExternalInput")
    d_tT = nc.dram_tensor("tT", [8, DSLOT], F32R, kind="ExternalInput")
    d_xT = nc.dram_tensor("xT", [4, XSLOT], F32R, kind="ExternalInput")
    d_wstat = nc.dram_tensor("wstat", [P, offs["WSCOLS"]], F32, kind="ExternalInput")
    d_wstatr = nc.dram_tensor("wstatr", [P, offs["WRCOLS"]], F32R, kind="ExternalInput")
    d_wbf = nc.dram_tensor("wbf", [P, offs["WBCOLS"]], BF, kind="ExternalInput")
    d_brow = nc.dram_tensor("brow", [1, offs["BRCOLS"]], F32R, kind="ExternalInput")
    d_wlay = [nc.dram_tensor(f"wlay{l}", [P, offs["WLCOLS"]], F32R,
                             kind="ExternalInput") for l in range(L)]
    d_scores = nc.dram_tensor("scores", [DSLOT], F32, kind="ExternalOutput")

    cc_tf_in = [nc.dram_tensor(f"cc_tf_in{l}", [DSLOT, D], BF) for l in range(L)]
    tf_tab = [nc.dram_tensor(f"tf_tab{l}", [TTAB, D], BF, addr_space="Shared")
              for l in range(L)]
    cc_xf_in = [nc.dram_tensor(f"cc_xf_in{l}", [XSLOT, D], BF) for l in range(L)]
    xf_tab = [nc.dram_tensor(f"xf_tab{l}", [XTAB, D], BF, addr_space="Shared")
              for l in range(L)]

    ident_f32 = nc.inline_tensor(np.eye(P, dtype=np.float32), name="ident_f32")
    ident_bf = nc.inline_tensor(np.eye(P, dtype=np.float32).astype(BF16),
                                name="ident_bf")
    ident_f8 = nc.inline_tensor(np.eye(P, dtype=np.float32).astype(FP8),
                                name="ident_f8")

    RG = [list(range(NCORES))]
    wso, wro, bro, bfo = offs["ws"], offs["wr"], offs["br"], offs["bf"]

    with tile.TileContext(nc) as tc:
        import contextlib
        ctx = contextlib.ExitStack()
        perm = ctx.enter_context(tc.tile_pool(name="perm", bufs=1))
        arot = ctx.enter_context(tc.tile_pool(name="arot", bufs=2))
        srot = ctx.enter_context(tc.tile_pool(name="srot", bufs=2))
        psA = ctx.enter_context(tc.tile_pool(name="psA", bufs=2, space="PSUM"))
        psMsg = ctx.enter_context(tc.tile_pool(name="psMsg", bufs=2, space="PSUM"))
        psCs = ctx.enter_context(tc.tile_pool(name="psCs", bufs=1, space="PSUM"))
        psSm = ctx.enter_context(tc.tile_pool(name="psSm", bufs=2, space="PSUM"))

        idx_t_sb = perm.tile([P, EP // 16], I16, tag="idx_t")
        idx_x_sb = perm.tile([P, EP // 16], I16, tag="idx_x")
        nm_t_sb = perm.tile([P, NT * P], F8, tag="nm_t")
        nm_x_sb = perm.tile([P, NT * P], F8, tag="nm_x")
        xs_sb = perm.tile([P, NT * P], BF, tag="xs")
        xsT_sb = perm.tile([P, EP], BF, tag="xsT")
        U_sb = perm.tile([P, H * DSLOT], BF, tag="U")
        wstat_sb = perm.tile([P, offs["WSCOLS"]], F32, tag="wstat")
        wstatr_sb = perm.tile([P, offs["WRCOLS"]], F32R, tag="wstatr")
        wbf_sb = perm.tile([P, offs["WBCOLS"]], BF, tag="wbf")
        brow_sb = perm.tile([1, offs["BRCOLS"]], F32R, tag="brow")
        wl_sb = perm.tile([P, offs["WLCOLS"]], F32R, tag="wl")
        tT_sb = perm.tile([8, DSLOT], F32R, tag="tT")
        xT_sb = perm.tile([4, XSLOT], F32R, tag="xT")
        tf_own = perm.tile([P, DSLOT], F32, tag="tf_own")
        tfT = perm.tile([P, DSLOT], F32R, tag="tfT")
        tn_sb = perm.tile([P, DSLOT], F32, tag="tn")
        rob = perm.tile([P, DSLOT], F32, tag="rob")
        tf_bf = perm.tile([P, DSLOT], BF, tag="tf_bf")
        xf_own = perm.tile([P, XSLOT], F32, tag="xf_own")
        xf_bf = perm.tile([P, XSLOT], BF, tag="xf_bf")
        xfT_bf = perm.tile([P, XSLOT], BF, tag="xfT_bf")
        id32 = perm.tile([P, P], F32, tag="id32")
        idbf = perm.tile([P, P], BF, tag="idbf")
        idf8 = perm.tile([P, P], F8, tag="idf8")
        ones_e = perm.tile([P, 1], BF, tag="ones_e")
        ones_rbf = perm.tile([1, P], BF, tag="ones_rbf")
        ln_tmp = perm.tile([P, 16], F32, tag="ln_tmp")
        eps5 = perm.tile([P, 1], F32, tag="eps5")
        sco = perm.tile([1, DSLOT], F32, tag="sco")

        sync, gps, te, act, dve = nc.sync, nc.gpsimd, nc.tensor, nc.scalar, nc.vector
        AF = mybir.ActivationFunctionType
        ALU = mybir.AluOpType

        sync.dma_start(out=idx_t_sb[:], in_=d_idx_t.ap())
        sync.dma_start(out=idx_x_sb[:], in_=d_idx_x.ap())
        sync.dma_start(out=nm_t_sb[:], in_=d_nm_t.ap())
        sync.dma_start(out=nm_x_sb[:], in_=d_nm_x.ap())
        sync.dma_start(out=wstat_sb[:], in_=d_wstat.ap())
        sync.dma_start(out=wstatr_sb[:], in_=d_wstatr.ap())
        sync.dma_start(out=wbf_sb[:], in_=d_wbf.ap())
        sync.dma_start(out=brow_sb[:1, :], in_=d_brow.ap())
        sync.dma_start(out=tT_sb[:8, :], in_=d_tT.ap())
        sync.dma_start(out=xT_sb[:4, :], in_=d_xT.ap())
        sync.dma_start(out=id32[:], in_=ident_f32.ap())
        sync.dma_start(out=idbf[:], in_=ident_bf.ap())
        sync.dma_start(out=idf8[:], in_=ident_f8.ap())
        dve.memset(ones_e[:], 1.0)
        dve.memset(ones_rbf[:1, :], 1.0)
        dve.memset(eps5[:], 1e-5)

        def wst(name, n):
            return wstat_sb[:, wso[name]:wso[name] + n]

        def wsr(name, n):
            return wstatr_sb[:, wro[name]:wro[name] + n]

        def bslice(name, n):
            return brow_sb[:1, bro[name]:bro[name] + n]

        def wbfs(name, n):
            return wbf_sb[:, bfo[name]:bfo[name] + n]

        def wls(loff, name, n):
            return wl_sb[:, loff[name]:loff[name] + n]

        ones_r32 = bslice("ones128", P).tensor[0:1, bro["ones128"]:bro["ones128"] + P] \
            if False else bslice("ones128", P)
        ones512 = bslice("ones512", DSLOT)

        # ---- phase 0: input projections
        def input_proj(src_sb, kdim, wname, bname, nslots, dst_sb):
            for i in range(nslots // P):
                ps = psA.tile([P, DSLOT], F32, tag="psa")
                te.matmul(out=ps[:, :P], lhsT=src_sb[:kdim, i * P:(i + 1) * P],
                          rhs=wsr(wname, P)[:kdim, :], start=True, stop=False)
                te.matmul(out=ps[:, :P], lhsT=ones_r32,
                          rhs=bslice(bname, P), start=False, stop=True)
                act.activation(dst_sb[:, i * P:(i + 1) * P], ps[:, :P], AF.Relu)

        input_proj(tT_sb, 8, "fct_w", "fct_b", DSLOT, tf_own)
        input_proj(xT_sb, 4, "fcx_w", "fcx_b", XSLOT, xf_own)

        def push_tf_table(l):
            dve.tensor_copy(out=tf_bf[:], in_=tf_own[:])
            gps.dma_start(
                out=cc_tf_in[l].ap().rearrange("(t p) c -> p t c", p=P),
                in_=tf_bf[:].rearrange("p (t c) -> p t c", c=P))
            gps.collective_compute(
                "AllGather", mybir.AluOpType.bypass, replica_groups=RG,
                ins=[cc_tf_in[l].ap()], outs=[tf_tab[l].ap()])

        def push_xf_table(l):
            dve.tensor_copy(out=xf_bf[:], in_=xf_own[:])
            gps.dma_start(
                out=cc_xf_in[l].ap().rearrange("(t p) c -> p t c", p=P),
                in_=xf_bf[:].rearrange("p (t c) -> p t c", c=P))
            gps.collective_compute(
                "AllGather", mybir.AluOpType.bypass, replica_groups=RG,
                ins=[cc_xf_in[l].ap()], outs=[xf_tab[l].ap()])

        push_tf_table(0)
        push_xf_table(0)

        # ---- layers
        for l in range(L):
            loff = offs["wl"][l]
            sync.dma_start(out=wl_sb[:], in_=d_wlay[l].ap())
            csum = psCs.tile([P, DSLOT], F32, tag="csum")

            for i in range(4):
                ps = psSm.tile([P, P], F32, tag="pss")
                te.transpose(out=ps[:], in_=tf_own[:, i * P:(i + 1) * P],
                             identity=id32[:])
                dve.tensor_copy(out=tfT[:, i * P:(i + 1) * P], in_=ps[:])

            for vi, v in enumerate(("tt", "xct")):
                # q, U
                for h in range(H):
                    psq = psA.tile([P, DSLOT], F32, tag="psa")
                    te.matmul(out=psq[:],
                              lhsT=wls(loff, f"wq_{v}", 512)[:, h * P:(h + 1) * P],
                              rhs=tfT[:], start=True, stop=False)
                    te.matmul(out=psq[:],
                              lhsT=bslice(f"bq_{l}{v}", 512)[:1, h * P:(h + 1) * P],
                              rhs=ones512, start=False, stop=True)
                    qT = srot.tile([P, DSLOT], F32R, tag="qT")
                    act.activation(qT[:], psq[:], AF.Copy)
                    psu = psA.tile([P, DSLOT], F32, tag="psa")
                    te.matmul(out=psu[:],
                              lhsT=wls(loff, f"wkT_{v}", 512)[:, h * P:(h + 1) * P],
                              rhs=qT[:], start=True, stop=True)
                    act.activation(U_sb[:, h * DSLOT:(h + 1) * DSLOT], psu[:],
                                   AF.Copy)

                # gathers
                tab = tf_tab[l] if v == "tt" else xf_tab[l]
                idx_sb = idx_t_sb if v == "tt" else idx_x_sb
                nm_sb = nm_t_sb if v == "tt" else nm_x_sb
                GSP = 768  # max safe dma_gather num_idxs (HW pitch limit ~1023)
                for s in range(EP // GSP):
                    gps.dma_gather(
                        out_ap=xs_sb[:, s * GSP:(s + 1) * GSP].rearrange(
                            "p (t c) -> p t c", c=P),
                        in_ap=tab.ap(),
                        idxs_ap=idx_sb[:, s * (GSP // 16):(s + 1) * (GSP // 16)],
                        num_idxs=GSP, num_idxs_reg=GSP, elem_size=P)
                    gps.dma_gather(
                        out_ap=xsT_sb[:, s * GSP:(s + 1) * GSP].rearrange(
                            "p (o e) -> p o e", o=1),
                        in_ap=tab.ap(),
                        idxs_ap=idx_sb[:, s * (GSP // 16):(s + 1) * (GSP // 16)],
                        num_idxs=GSP, num_idxs_reg=GSP, elem_size=P, transpose=True)

                Uap = U_sb[:].rearrange("p (h d) -> p d h", h=H)

                for k in range(NCHUNK):
                    a_ch = arot.tile([P, TPC * P], BF, tag="a_ch")
                    msg = psMsg.tile([P, P + 4], F32, tag="msg")
                    Uk = Uap[:, k * CHD:(k + 1) * CHD, :]
                    for g, gn in ((0, 4), (1, 4), (2, 1)):
                        g0 = g * 4
                        lg = psA.tile([P, 4 * P], F32, tag="psa")
                        te.matmul(out=lg[:, :gn * P], lhsT=idf8[:],
                                  rhs=nm_sb[:, (k * TPC + g0) * P:(k * TPC + g0 + gn) * P],
                                  start=True, stop=False)
                        for t in range(gn):
                            te.matmul(out=lg[:, t * P:(t + 1) * P],
                                      lhsT=xsT_sb[:, (k * TPC + g0 + t) * P:(k * TPC + g0 + t + 1) * P],
                                      rhs=Uk, start=False, stop=(t == gn - 1))
                        act.activation(a_ch[:, g0 * P:(g0 + gn) * P],
                                       lg[:, :gn * P], AF.Exp)
                    for t in range(TPC):
                        a_t = a_ch[:, t * P:(t + 1) * P]
                        te.matmul(out=msg[:, P:P + 1], lhsT=a_t, rhs=ones_e[:],
                                  start=(t == 0), stop=False)
                        te.matmul(out=msg[:, 0:P], lhsT=a_t,
                                  rhs=xs_sb[:, (k * TPC + t) * P:(k * TPC + t + 1) * P],
                                  start=False, stop=(t == TPC - 1))
                    # epilogue
                    rec = srot.tile([P, 2], F32, tag="rec")
                    dve.tensor_scalar_add(out=rec[:, 0:1], in0=msg[:, P:P + 1],
                                          scalar1=1e-16)
                    dve.reciprocal(out=rec[:, 1:2], in_=rec[:, 0:1])
                    mdiv = srot.tile([P, P], BF, tag="mdiv")
                    act.activation(mdiv[:], msg[:, 0:P], AF.Copy,
                                   scale=rec[:, 1:2])
                    tps = psSm.tile([P, P], BF, tag="pss")
                    te.transpose(out=tps[:], in_=mdiv[:], identity=idbf[:])
                    mdT = srot.tile([P, P], BF, tag="mdT")
                    dve.tensor_copy(out=mdT[:], in_=tps[:])
                    mdTh = mdT[:].rearrange("p (d h) -> p h d", h=H)
                    for h in range(H):
                        te.matmul(
                            out=csum[(k % 4) * CHD:(k % 4) * CHD + CHD,
                                     (k // 4) * P:(k // 4) * P + P],
                            lhsT=mdTh[:, h, :],
                            rhs=wbfs(f"wv4_{l}{v}", 512)[:, h * P:(h + 1) * P],
                            start=(vi == 0 and k < 4 and h == 0), stop=False,
                            tile_position=(0, (k % 4) * CHD))
                # skip connection + conv bias
                for i in range(4):
                    te.matmul(out=csum[:, i * P:(i + 1) * P],
                              lhsT=tfT[:, i * P:(i + 1) * P],
                              rhs=wls(loff, f"ws_{v}", P),
                              start=False, stop=False)
                    te.matmul(out=csum[:, i * P:(i + 1) * P],
                              lhsT=ones_r32,
                              rhs=bslice(f"bconv_{l}{v}", P),
                              start=False, stop=(vi == 1 and i == 3))

            # tn = relu(tf + csum)
            dve.tensor_add(out=tn_sb[:], in0=tf_own[:], in1=csum[:])
            act.activation(tn_sb[:], tn_sb[:], AF.Relu)

            # ob = LN(relu(tn @ Wout + bout)) * g + beta
            for i in range(4):
                ps = psSm.tile([P, P], F32, tag="pss")
                te.transpose(out=ps[:], in_=tn_sb[:, i * P:(i + 1) * P],
                             identity=id32[:])
                dve.tensor_copy(out=tfT[:, i * P:(i + 1) * P], in_=ps[:])  # tnT
            for i in range(4):
                ps = psA.tile([P, DSLOT], F32, tag="psa")
                te.matmul(out=ps[:, :P], lhsT=tfT[:, i * P:(i + 1) * P],
                          rhs=wsr(f"wout{l}", P), start=True, stop=False)
                te.matmul(out=ps[:, :P], lhsT=ones_r32,
                          rhs=bslice(f"bout_{l}", P), start=False, stop=True)
                act.activation(rob[:, i * P:(i + 1) * P], ps[:, :P], AF.Relu)
            mu = ln_tmp[:, 0:4]
            msq = ln_tmp[:, 4:8]
            rstd = ln_tmp[:, 8:12]
            nmr = ln_tmp[:, 12:16]
            dve.tensor_reduce(out=mu, in_=rob[:].rearrange("p (t c) -> p t c", c=P),
                              axis=mybir.AxisListType.X, op=ALU.add)
            dve.tensor_scalar_mul(out=mu, in0=mu, scalar1=1.0 / P)
            sq = srot.tile([P, DSLOT], F32, tag="qT")
            dve.tensor_mul(out=sq[:], in0=rob[:], in1=rob[:])
            dve.tensor_reduce(out=msq, in_=sq[:].rearrange("p (t c) -> p t c", c=P),
                              axis=mybir.AxisListType.X, op=ALU.add)
            dve.tensor_scalar_mul(out=msq, in0=msq, scalar1=1.0 / P)
            dve.tensor_mul(out=rstd, in0=mu, in1=mu)
            dve.tensor_tensor(out=rstd, in0=msq, in1=rstd, op=ALU.subtract)
            act.activation(rstd, rstd, AF.Sqrt, bias=eps5[:, 0:1])
            dve.reciprocal(out=rstd, in_=rstd)
            dve.tensor_mul(out=nmr, in0=mu, in1=rstd)
            dve.tensor_scalar_mul(out=nmr, in0=nmr, scalar1=-1.0)
            for i in range(4):
                sl = slice(i * P, (i + 1) * P)
                dve.tensor_scalar(out=rob[:, sl], in0=rob[:, sl],
                                  scalar1=rstd[:, i:i + 1], scalar2=nmr[:, i:i + 1],
                                  op0=ALU.mult, op1=ALU.add)
                dve.tensor_mul(out=rob[:, sl], in0=rob[:, sl], in1=wst(f"gb{l}", P))
                dve.tensor_add(out=rob[:, sl], in0=rob[:, sl], in1=wst(f"bb{l}", P))
            dve.tensor_add(out=tf_own[:], in0=tn_sb[:], in1=rob[:])
            if l < L - 1:
                push_tf_table(l + 1)

            # xf MLP (skipped for the last layer; xf_l+1 is unused there)
            if l < L - 1:
                dve.tensor_copy(out=xf_bf[:], in_=xf_own[:])
                for i in range(XSLOT // P):
                    ps = psSm.tile([P, P], BF, tag="pss")
                    te.transpose(out=ps[:], in_=xf_bf[:, i * P:(i + 1) * P],
                                 identity=idbf[:])
                    dve.tensor_copy(out=xfT_bf[:, i * P:(i + 1) * P], in_=ps[:])
                for j in range(XSLOT // DSLOT):
                    ps = psA.tile([P, DSLOT], F32, tag="psa")
                    te.matmul(out=ps[:], lhsT=wbfs(f"w1_{l}", P),
                              rhs=xfT_bf[:, j * DSLOT:(j + 1) * DSLOT],
                              start=True, stop=True)
                    act.activation(xfT_bf[:, j * DSLOT:(j + 1) * DSLOT], ps[:],
                                   AF.Relu, bias=wst(f"b1_{l}", 1))
                for i in range(XSLOT // P):
                    ps = psSm.tile([P, P], F32, tag="pss")
                    te.matmul(out=ps[:], lhsT=xfT_bf[:, i * P:(i + 1) * P],
                              rhs=wbfs(f"w2_{l}", P), start=True, stop=False)
                    te.matmul(out=ps[:], lhsT=ones_rbf[:1, :],
                              rhs=wbfs(f"b2r_{l}", P)[:1, :],
                              start=False, stop=True)
                    dve.tensor_add(out=xf_own[:, i * P:(i + 1) * P],
                                   in0=xf_own[:, i * P:(i + 1) * P], in1=ps[:])
                push_xf_table(l + 1)

        # ---- scores
        for i in range(4):
            ps = psSm.tile([P, P], F32, tag="pss")
            te.transpose(out=ps[:], in_=tf_own[:, i * P:(i + 1) * P],
                         identity=id32[:])
            dve.tensor_copy(out=tfT[:, i * P:(i + 1) * P], in_=ps[:])
        psc = psA.tile([P, DSLOT], F32, tag="psa")
        te.matmul(out=psc[:1, :], lhsT=wsr("fcout_w", 1), rhs=tfT[:],
                  start=True, stop=False)
        te.matmul(out=psc[:1, :], lhsT=bslice("fcout_b", 1),
                  rhs=ones512, start=False, stop=True)
        act.activation(sco[:1, :], psc[:1, :], AF.Copy)
        sync.dma_start(out=d_scores.ap()[None, :], in_=sco[:1, :])
        ctx.close()
    nc.compile()
    return nc


_prog_cache = {}


def _get_program(offsets):
    key = "v1"
    if key not in _prog_cache:
        _prog_cache[key] = build_program(offsets)
    return _prog_cache[key]


last_result = None
last_run_s = None


def kernel(x, t, e_t_src, e_t_dst, e_xct_src, e_xct_dst, unique_mask, params):
    global last_result
    in_maps, offsets = _prep_inputs(x, t, e_t_src, e_t_dst,
                                    e_xct_src, e_xct_dst, params)
    nc = _get_program(offsets)
    import time as _time
    t0 = _time.time()
    res = bass_utils.run_bass_kernel_spmd(nc, in_maps, core_ids=list(range(NCORES)))
    global last_run_s
    last_run_s = _time.time() - t0
    last_result = res
    out = np.zeros((BATCH, RR), np.float32)
    for c in range(NCORES):
        out[2 * c:2 * c + 2] = res.results[c]["scores"][:NDST].reshape(2, RR)
    mask = np.asarray(unique_mask, bool)
    return np.where(mask, out, np.float32(-np.inf)).astype(np.float32)
